# revision 2
# baseline (speedup 1.0000x reference)
"""Trainium2 Bass kernel: 5x5 window median+variance denoise filter.

y = relu(x - noise_var/(var5x5(x)+1e-10) * (x - median5x5(x) + noise_bias))
with zero-padded 5x5 windows, unbiased variance (ddof=1).

Sharding: pure data parallel, B=16 images split 2-per-core across 8 cores.

v2: fp16 datapath. The median comparator network runs on DVE in fp16 to hit
the 2x_1p perf mode (all operands 2-byte, stride-1, 4B-aligned -- odd column
offsets are re-aligned via ACT-engine copies so no network op falls back to
1x). Squares, view-alignment copies, dtype conversions and the final relu
run on the otherwise-idle ACT engine. Variance accumulates in fp32 where it
matters (horizontal s25 sum, d/reciprocal path). Host pre-pads and converts
x to fp16; output returns fp16 and is upcast on host. Total numeric error
~1e-3 rel vs the 2e-2 harness gate.

Median via a pruned comparator network with shared column sorts:
  sort5 over the 5 dy-shifted planes (9 CE, shared by 5 horizontal windows)
  T = odd-even merge of adjacent sorted columns (13 CE, shared by 2 windows)
  final rank-12 selection from T(x-2), T(x), S(x+2) (35 CE, single-sided
  min/max pruned) -- 90 DVE min/max ops per full-image sweep, verified
  offline by exhaustive 0-1 principle.
"""
import numpy as np

import concourse.bass as bass  # noqa: F401
import concourse.mybir as mybir
from concourse import bacc, tile
from concourse.bass_utils import run_bass_kernel_spmd

F32 = mybir.dt.float32
F16 = mybir.dt.float16
ALU = mybir.AluOpType
ACTF = mybir.ActivationFunctionType

# (i, j, need_min, need_max) per structure; designed + 0/1-verified offline.
SORT5 = [(0, 1, 1, 1), (3, 4, 1, 1), (2, 4, 1, 1), (2, 3, 1, 1), (0, 3, 1, 1),
         (0, 2, 1, 1), (1, 4, 1, 1), (1, 3, 1, 1), (1, 2, 1, 1)]
T_CES = [(0, 5, 1, 1), (4, 9, 1, 1), (4, 5, 1, 1), (2, 7, 1, 1), (2, 4, 1, 1),
         (7, 5, 1, 1), (1, 6, 1, 1), (3, 8, 1, 1), (3, 6, 1, 1), (1, 2, 1, 1),
         (3, 4, 1, 1), (6, 7, 1, 1), (8, 5, 1, 1)]
F_CES = [(0, 10, 0, 1), (5, 15, 1, 0), (5, 10, 1, 1), (4, 14, 1, 1),
         (4, 5, 0, 1), (14, 10, 1, 0), (2, 12, 0, 1), (7, 17, 1, 0),
         (7, 12, 1, 1), (7, 5, 0, 1), (12, 14, 1, 1), (1, 11, 0, 1),
         (9, 19, 1, 0), (9, 11, 1, 1), (6, 16, 1, 1), (6, 9, 0, 1),
         (16, 11, 1, 0), (3, 13, 0, 1), (8, 18, 1, 0), (8, 13, 1, 1),
         (8, 9, 1, 1), (13, 16, 1, 0), (8, 5, 1, 1), (9, 12, 1, 1),
         (13, 14, 1, 1), (8, 20, 0, 1), (13, 24, 1, 0), (13, 20, 0, 1),
         (9, 22, 0, 1), (22, 20, 1, 0), (5, 21, 0, 1), (14, 21, 1, 0),
         (12, 23, 1, 0), (12, 14, 0, 1), (14, 22, 1, 0)]
F_OUT = 14

H = 512
W = 512
IMGS_PER_CORE = 2
N_CORES = 8
WIDE = W + 4          # 2-col halo each side
NBUF = 64             # cap on SBUF working buffers of [128, 2, WIDE] f16
NVB_COLS = 4          # nv, nb, 1/(24 nv), 1e-10/nv


class BufPool:
    """Free-list over preallocated fixed SBUF tensors. Tile's dependency
    tracker makes reuse safe (WAR/RAW serialization on the same tensor)."""

    def __init__(self, nc):
        self.nc = nc
        self.bufs = []
        self.free = []

    def alloc(self):
        if self.free:
            return self.free.pop()
        idx = len(self.bufs)
        assert idx < NBUF, "SBUF buffer pool exhausted"
        t = self.nc.alloc_sbuf_tensor(f"wb{idx}", [128, 2, WIDE], F16).ap()
        self.bufs.append(t)
        return t

    def release(self, t):
        self.free.append(t)


class Wire:
    """SSA value living at column offset `off` of `buf`."""

    def __init__(self, buf, off, owned, pool, on_die=None):
        self.buf = buf
        self.off = off
        self.owned = owned      # release buf to pool when dead
        self.pool = pool
        self.reads_left = 0
        self.on_die = on_die

    def ap(self, width):
        return self.buf[:, :, self.off:self.off + width]

    def read_done(self):
        self.reads_left -= 1
        if self.reads_left == 0:
            self._die()

    def read_done_zero(self):
        if self.reads_left == 0:
            self._die()

    def _die(self):
        if self.owned:
            self.pool.release(self.buf)
        if self.on_die is not None:
            self.on_die()

    def detach_views(self, n_views):
        """Transfer buffer ownership to n_views future views; returns the
        on_die callback the views share. Call read_done() after to consume
        the terminal hold."""
        buf, owned, pool = self.buf, self.owned, self.pool
        self.owned = False
        state = {"n": n_views}

        def on_die():
            state["n"] -= 1
            if state["n"] == 0 and owned:
                pool.release(buf)
        return on_die


def run_stage(nc, pool, wires, ces, width, terminal_reads):
    """Emit one structure stage. A position's lifetime is split into segments
    at each rewrite; each Wire object gets the read count of its own segment
    so buffers release as soon as truly dead."""
    n = len(wires)
    # segs[i] = read counts per segment of position i (segment ends at a
    # write of i, which itself reads the old value).
    segs = [[] for _ in range(n)]
    cur = [0] * n
    for (a, b, nmin, nmax) in ces:
        cur[a] += 1
        cur[b] += 1
        if nmin:
            segs[a].append(cur[a])
            cur[a] = 0
        if nmax:
            segs[b].append(cur[b])
            cur[b] = 0
    for i in range(n):
        segs[i].append(cur[i] + terminal_reads.get(i, 0))

    seg_idx = [0] * n
    for i in range(n):
        wires[i].reads_left += segs[i][0]
        if segs[i][0] == 0:
            wires[i].read_done_zero()

    for (i, j, nmin, nmax) in ces:
        wi, wj = wires[i], wires[j]
        a = wi.ap(width)
        b = wj.ap(width)
        if nmin:
            lo = pool.alloc()
            nc.vector.tensor_tensor(lo[:, :, 0:width], a, b, ALU.min)
        if nmax:
            hi = pool.alloc()
            nc.vector.tensor_tensor(hi[:, :, 0:width], a, b, ALU.max)
        wi.read_done()
        wj.read_done()
        if nmin:
            seg_idx[i] += 1
            cnt = segs[i][seg_idx[i]]
            assert cnt > 0, "dead write (should be pruned offline)"
            wires[i] = Wire(lo, 0, True, pool)
            wires[i].reads_left = cnt
        if nmax:
            seg_idx[j] += 1
            cnt = segs[j][seg_idx[j]]
            assert cnt > 0, "dead write (should be pruned offline)"
            wires[j] = Wire(hi, 0, True, pool)
            wires[j].reads_left = cnt


def emit_chunk(nc, pool, f32bufs, tin, sq, out_tile, xa, ya, scal, img, half):
    r0 = half * 256
    full = lambda t: t[:, :, :]

    # ---- loads: 5 dy-shifted fp16 tiles [128, 2, WIDE] from the pre-padded
    # shard (rows/cols already carry the 2-wide zero halo) ----
    for k, dy in enumerate(range(-2, 3)):
        for b in range(2):
            s = img * (H + 4) + r0 + b * 128 + dy + 2
            nc.sync.dma_start(tin[k][:, b, :], xa[s: s + 128, :])

    # ---- squares on ACT (parallel with DVE) ----
    for k in range(5):
        nc.scalar.square(full(sq[k]), full(tin[k]))

    # ---- vertical window sums, fp16 2x ----
    V1 = pool.alloc()
    t01 = pool.alloc()
    nc.vector.tensor_tensor(full(t01), full(tin[0]), full(tin[1]), ALU.add)
    nc.vector.tensor_tensor(full(V1), full(tin[2]), full(tin[3]), ALU.add)
    nc.vector.tensor_tensor(full(V1), full(V1), full(t01), ALU.add)
    nc.vector.tensor_tensor(full(V1), full(V1), full(tin[4]), ALU.add)
    V2 = pool.alloc()
    nc.vector.tensor_tensor(full(t01), full(sq[0]), full(sq[1]), ALU.add)
    nc.vector.tensor_tensor(full(V2), full(sq[2]), full(sq[3]), ALU.add)
    nc.vector.tensor_tensor(full(V2), full(V2), full(t01), ALU.add)
    nc.vector.tensor_tensor(full(V2), full(V2), full(sq[4]), ALU.add)
    pool.release(t01)

    # ---- horizontal 5-sums ----
    # s25 in fp32: ACT upconverts V1, then pairwise adds on DVE (1x fp32).
    V1f, hA, s25, tt, dd = f32bufs
    nc.scalar.copy(full(V1f), full(V1))
    pool.release(V1)
    nc.vector.tensor_tensor(hA[:, :, 0:515], V1f[:, :, 0:515],
                            V1f[:, :, 1:516], ALU.add)
    nc.vector.tensor_tensor(s25[:, :, 0:513], hA[:, :, 0:513],
                            hA[:, :, 2:515], ALU.add)
    nc.vector.tensor_tensor(s25[:, :, 0:W], s25[:, :, 0:W],
                            V1f[:, :, 4:4 + W], ALU.add)
    # q25 in fp16 (first op odd-offset -> 1x, rest 2x), final op fp32 out.
    qh = pool.alloc()
    nc.vector.tensor_tensor(qh[:, :, 0:515], V2[:, :, 0:515],
                            V2[:, :, 1:516], ALU.add)
    qs = pool.alloc()
    nc.vector.tensor_tensor(qs[:, :, 0:513], qh[:, :, 0:513],
                            qh[:, :, 2:515], ALU.add)
    pool.release(qh)
    # q25 -> fp32 buffer (reuse hA): q25 = qs + V2(x+4)
    nc.vector.tensor_tensor(hA[:, :, 0:W], qs[:, :, 0:W],
                            V2[:, :, 4:4 + W], ALU.add)
    pool.release(qs)
    pool.release(V2)

    # ---- d = q25 - s25^2/25 ; dd = d/(24 nv) + 1e-10/nv ; rcp = 1/dd ----
    nv_ap, nb_ap, c1_ap, c2_ap = scal
    nc.vector.tensor_tensor(tt[:, :, 0:W], s25[:, :, 0:W], s25[:, :, 0:W],
                            ALU.mult)
    nc.vector.scalar_tensor_tensor(tt[:, :, 0:W], tt[:, :, 0:W], -1.0 / 25.0,
                                   hA[:, :, 0:W], ALU.mult, ALU.add)
    nc.vector.tensor_scalar(dd[:, :, 0:W], tt[:, :, 0:W], c1_ap, c2_ap,
                            ALU.mult, ALU.add)
    nc.vector.reciprocal_approx_fast(out=tt[:, :, 0:W], in_=dd[:, :, 0:W])
    rcp = pool.alloc()     # fp16 copy of reciprocal for the 2x formula ops
    nc.scalar.copy(rcp[:, :, 0:W], tt[:, :, 0:W])

    # ---- median network (all fp16, all operands 4B-aligned) ----
    s_wires = [Wire(tin[k], 0, False, pool) for k in range(5)]
    run_stage(nc, pool, s_wires, SORT5, WIDE, {k: 2 for k in range(5)})

    # Sorted column planes r_k. Views: A_k = r_k @0 (width 515) and
    # C_k = r_k @4 (width 512) stay in place; B_k = r_k @1 is copied by ACT
    # into an aligned buffer so every T op keeps the 2x perf mode.
    t_wires = [None] * 10
    c_views = [None] * 5
    for k in range(5):
        rk = s_wires[k]
        bk = pool.alloc()
        nc.scalar.copy(bk[:, :, 0:515], rk.buf[:, :, rk.off + 1:rk.off + 516])
        rk.read_done()      # the ACT copy consumed one terminal hold
        od = rk.detach_views(2)
        t_wires[k] = Wire(rk.buf, rk.off + 0, False, pool, on_die=od)
        c_views[k] = Wire(rk.buf, rk.off + 4, False, pool, on_die=od)
        t_wires[k + 5] = Wire(bk, 0, True, pool)
        rk.read_done()      # consume second terminal hold

    run_stage(nc, pool, t_wires, T_CES, W + 3, {j: 1 for j in range(10)})

    f_wires = [None] * 25
    for j in range(10):
        tw = t_wires[j]
        od = tw.detach_views(2)
        f_wires[j] = Wire(tw.buf, tw.off + 0, False, pool, on_die=od)
        f_wires[j + 10] = Wire(tw.buf, tw.off + 2, False, pool, on_die=od)
        tw.read_done()
    for k in range(5):
        f_wires[20 + k] = c_views[k]

    run_stage(nc, pool, f_wires, F_CES, W, {F_OUT: 1})
    mid = f_wires[F_OUT]

    # ---- formula: y = relu(x - rcp*((x + nb) - mid)), all fp16 2x ----
    xc = tin[2][:, :, 2:2 + W]              # center plane = x
    u = pool.alloc()
    nc.vector.scalar_tensor_tensor(u[:, :, 0:W], xc, nb_ap, mid.ap(W),
                                   ALU.add, ALU.subtract)
    mid.read_done()
    nc.vector.tensor_tensor(u[:, :, 0:W], rcp[:, :, 0:W], u[:, :, 0:W],
                            ALU.mult)
    pool.release(rcp)
    nc.vector.tensor_tensor(u[:, :, 0:W], xc, u[:, :, 0:W], ALU.subtract)
    nc.scalar.activation(out_tile[:, :, :], u[:, :, 0:W], ACTF.Relu)
    pool.release(u)

    # ---- store (fp16) ----
    for b in range(2):
        nc.sync.dma_start(
            ya[img * H + r0 + b * 128: img * H + r0 + b * 128 + 128, :],
            out_tile[:, b, :],
        )


def build_module(repeat=1, hw_loop=None):
    nc = bacc.Bacc(
        "TRN2",
        target_bir_lowering=False,
        debug=False,
        enable_asserts=False,
        num_devices=N_CORES,
    )
    x = nc.dram_tensor("x", [IMGS_PER_CORE, H + 4, WIDE], F16,
                       kind="ExternalInput")
    nvb = nc.dram_tensor("nvb", [128, NVB_COLS], F32, kind="ExternalInput")
    y = nc.dram_tensor("y", [IMGS_PER_CORE, H, W], F16, kind="ExternalOutput")

    xa = x.ap().flatten_outer_dims()    # [2*516, 516] fp16
    ya = y.ap().flatten_outer_dims()

    with tile.TileContext(nc) as tc:
        pool = BufPool(nc)
        nvb_t = nc.alloc_sbuf_tensor("nvb_t", [128, NVB_COLS], F32).ap()
        nc.sync.dma_start(nvb_t[:, :], nvb.ap()[:, :])
        scal = tuple(nvb_t[:, i:i + 1] for i in range(4))

        # double-buffered input/square/output tiles (chunk parity)
        tin = [[nc.alloc_sbuf_tensor(f"tin{p}_{k}", [128, 2, WIDE], F16).ap()
                for k in range(5)] for p in range(2)]
        sq = [[nc.alloc_sbuf_tensor(f"sq{p}_{k}", [128, 2, WIDE], F16).ap()
               for k in range(5)] for p in range(2)]
        out_t = [nc.alloc_sbuf_tensor(f"out{p}", [128, 2, W], F16).ap()
                 for p in range(2)]
        f32bufs = tuple(
            nc.alloc_sbuf_tensor(f"f32_{n}", [128, 2, WIDE], F32).ap()
            for n in ("V1f", "hA", "s25", "tt", "dd"))

        def body():
            for _ in range(repeat):
                for ci in range(2 * IMGS_PER_CORE):
                    img, half = divmod(ci, 2)
                    p = ci & 1
                    emit_chunk(nc, pool, f32bufs, tin[p], sq[p], out_t[p],
                               xa, ya, scal, img, half)

        if hw_loop is None:
            body()
        else:
            with tc.For_i(0, hw_loop, 1):
                body()

    nc.compile()
    return nc


_MODULE = None


def _get_module():
    global _MODULE
    if _MODULE is None:
        _MODULE = build_module()
    return _MODULE


def make_in_maps(x, nv, nb):
    """Host-side prep: pad + fp16-convert x, build per-core input maps."""
    nvb = np.empty((128, NVB_COLS), np.float32)
    nvb[:, 0] = nv
    nvb[:, 1] = nb
    nvb[:, 2] = 1.0 / (24.0 * nv)
    nvb[:, 3] = 1e-10 / nv

    B = x.shape[0]
    xpad = np.zeros((B, H + 4, WIDE), np.float16)
    xpad[:, 2:2 + H, 2:2 + W] = x[:, 0]
    in_maps = []
    for c in range(N_CORES):
        shard = np.ascontiguousarray(
            xpad[c * IMGS_PER_CORE:(c + 1) * IMGS_PER_CORE])
        in_maps.append({"x": shard, "nvb": nvb})
    return in_maps


def kernel(x, noise_var, noise_bias):
    x = np.ascontiguousarray(np.asarray(x, dtype=np.float32))
    nv = float(np.asarray(noise_var).reshape(-1)[0])
    nb = float(np.asarray(noise_bias).reshape(-1)[0])
    B = x.shape[0]
    assert x.shape == (B, 1, H, W) and B == N_CORES * IMGS_PER_CORE

    nc = _get_module()
    in_maps = make_in_maps(x, nv, nb)
    res = run_bass_kernel_spmd(nc, in_maps, core_ids=list(range(N_CORES)))
    y = np.empty((B, 1, H, W), np.float32)
    for c in range(N_CORES):
        y[c * IMGS_PER_CORE:(c + 1) * IMGS_PER_CORE, 0] = \
            res.results[c]["y"].astype(np.float32)
    return y
